# revision 5
# baseline (speedup 1.0000x reference)
"""Haar DWT (NCHW) on 8 Trainium2 NeuronCores.

Full input x: (8, 64, 512, 512) f32 -> output (8, 256, 256, 256) f32
(concat of LL, LH, HL, HH subbands along channel dim, each spatially halved).

Sharding: batch dim across the 8 cores (1 image of 64x512x512 per core,
no communication).

Per-core pipeline (256 tiles of [128 rows x 512 cols], i.e. one
(channel, row-quarter) block per tile):
  1. DMA in: 256 KiB contiguous rows -> SBUF [128, 512]
  2. PE matmul with a constant 128x128 block-pair matrix: computes the
     vertical (row) Haar butterfly 0.5*(even +/- odd rows) -> PSUM.
     The 0.5 scale is folded into the weights.
  3. ScalarE copies PSUM -> SBUF (ScalarE sits close to PSUM).
  4. VectorE does the horizontal (column) butterfly with two
     tensor_tensor ops on stride-2 views (even/odd columns).
  5. DMA out: one strided DMA scatters the tile's 4 subband chunks to
     their regions of the output tensor.
"""

import sys

sys.path.insert(0, "/opt/trn_rl_repo")

import numpy as np

import concourse.bass as bass
import concourse.bacc as bacc
import concourse.mybir as mybir
from concourse import tile
from concourse.bass_utils import run_bass_kernel_spmd

N_CORES = 8
C = 64          # channels per core
H = 512
W = 512
HO = H // 2     # 256
WO = W // 2     # 256
P = 128         # SBUF partitions / rows per tile
QUARTERS = H // P  # 4 row-blocks per channel

F32 = mybir.dt.float32


def _haar_row_weights() -> np.ndarray:
    """[K=128, M=128]: out[m] = sum_k W[k, m] * row[k].

    m in [0,64):  0.5*(row 2m + row 2m+1)   (vertical lowpass,  rs)
    m in [64,128): 0.5*(row 2m' +1 - row 2m') (vertical highpass, rd)
    """
    w = np.zeros((P, P), dtype=np.float32)
    for m in range(64):
        w[2 * m, m] = 0.5
        w[2 * m + 1, m] = 0.5
        w[2 * m, 64 + m] = -0.5
        w[2 * m + 1, 64 + m] = 0.5
    return w


def build_nc() -> bass.Bass:
    nc = bacc.Bacc()
    x = nc.dram_tensor("x", [C, H, W], F32, kind="ExternalInput")
    out = nc.dram_tensor("out", [4 * C, HO, WO], F32, kind="ExternalOutput")
    w_dram = nc.inline_tensor(_haar_row_weights(), name="haar_w")

    # out channel = sb*64 + c with sb = ph + 2*half  (ph: rs/rd partition
    # half, half: even/odd column result). Decompose the 256-channel dim so
    # the store AP can address (half, ph, c) directly.
    out_v = out.rearrange("(half ph c) h w -> half ph c h w", half=2, ph=2)

    with tile.TileContext(nc) as tc:
        with (
            tc.tile_pool(name="const", bufs=1) as cpool,
            tc.tile_pool(name="pin", bufs=6) as pin,
            tc.tile_pool(name="psum", bufs=6, space=bass.MemorySpace.PSUM) as ppsum,
            tc.tile_pool(name="pmid", bufs=6) as pmid,
            tc.tile_pool(name="pout", bufs=6) as pout,
        ):
            w_sb = cpool.tile([P, P], F32)
            nc.sync.dma_start(w_sb[:], w_dram[:])

            for c in range(C):
                for q in range(QUARTERS):
                    in_sb = pin.tile([P, W], F32)
                    nc.sync.dma_start(in_sb[:], x[c, P * q : P * (q + 1), :])

                    ps = ppsum.tile([P, W], F32)
                    nc.tensor.matmul(ps[:], w_sb[:], in_sb[:], start=True, stop=True)

                    mid = pmid.tile([P, W], F32)
                    nc.scalar.copy(mid[:], ps[:])

                    o_sb = pout.tile([P, W], F32)
                    m3 = mid[:].rearrange("p (w two) -> p w two", two=2)
                    # cols 0:256 = even+odd col  (LL rows on p<64, LH on p>=64)
                    nc.vector.tensor_add(o_sb[:, 0:WO], m3[:, :, 0], m3[:, :, 1])
                    # cols 256:512 = odd-even col (HL rows on p<64, HH on p>=64)
                    nc.vector.tensor_sub(o_sb[:, WO:W], m3[:, :, 1], m3[:, :, 0])

                    # DMA APs are limited to 3 dims — one store per rs/rd
                    # partition half (ph selects LL/HL vs LH/HH channels).
                    for ph in range(2):
                        src = o_sb[64 * ph : 64 * (ph + 1), :].rearrange(
                            "pl (half w) -> pl half w", half=2
                        )
                        # [half, h=64, w] -> [h, half, w] to match src order
                        dst = out_v[:, ph, c, 64 * q : 64 * (q + 1), :].transpose(
                            [1, 0, 2]
                        )
                        nc.sync.dma_start(dst, src)

    # Bacc defers register allocation to finalize()/compile() — the
    # spmd/pjrt path serializes the BIR as-is, so finalize here.
    nc.finalize()
    return nc


_NC_CACHE: dict = {}


def _get_nc() -> bass.Bass:
    if "nc" not in _NC_CACHE:
        _NC_CACHE["nc"] = build_nc()
    return _NC_CACHE["nc"]


def kernel(x: np.ndarray) -> np.ndarray:
    x = np.asarray(x)
    assert x.shape == (N_CORES, C, H, W), x.shape
    nc = _get_nc()
    in_maps = [{"x": np.ascontiguousarray(x[i])} for i in range(N_CORES)]
    res = run_bass_kernel_spmd(nc, in_maps, list(range(N_CORES)))
    return np.stack([res.results[i]["out"] for i in range(N_CORES)], axis=0)
